# revision 54
# baseline (speedup 1.0000x reference)
"""Causal single-head attention (b=4, s=2048, d=1024, h=64) on 8 TRN2 cores.

Sharding: core c -> (batch b = c//2, g = c%2); the core owns the balanced
q-chunk pair A,B = (0,3) if g==0 else (1,2) (512 queries each).  Permuted
row order [A; B; rest0; rest1] makes one uniform SPMD program valid for
all 8 cores (cross-chunk gating via exp biases in {0, -40}).

fp8 DoubleRow pipeline:
  - x is shipped as fp8 xa (+ an fp8 residual xb0 for the first 256 keys
    of chunk A, protecting the low-N_eff early rows' v precision).
  - K/Q projected with DR matmuls to PSUM partitions 0:64 (kt/qd bf16,
    weights host-scaled x32; the combined scale lands in the exp scale;
    bv is folded in on the host after the division).
  - V projected directly in transposed [keys, h] layout (x stationary,
    Wva+Wvb double-quantized fp8 weights accumulated into one psum), then
    evacuated to bf16 vbf and double-quantized to fp8 a+b in SBUF ->
    vext [128k, blk, a|b].
  - scores bf16; one exp per 2-k-block pair (f32 psum -> fp8 pt; bf16 pt
    for the precision-critical slot0-diag pairs p1/p2, which also use
    bf16 plain AV matmuls against vbf0).  Diag pairs trim the causal
    triangle: pair t=0 widths (512,384), t=1 (256,128), packed flat.
  - diag masks via gpsimd affine_select on pt (keep p<=f).
  - AV: fp8 DR matmul per pair -> po[s]=[A;B] partials; denominators via
    DR ones-matmuls into ONE pd bank (slot0's ones live in stationary
    cols 0:32, slot1's in 32:64, each zero-padded so both slots share
    the [64, 512] accumulator).  Host: out = (A+B)/den/32 + bv.
  - PSUM: sc pool (2 x 2-bank score-pair tiles, double-buffered) + pj
    pool (1 bank serializing K/V/corr projections; Q borrows sc bufs) +
    po (2 banks) + pd (1) = 8 banks.
  - PE warm-up matmuls hold the p-state ramp until the first x DMA lands.
  - Software-pipelined emission: scores run 2 pairs ahead of AVs so the
    in-order PE queue never blocks on exp latency.
"""

import numpy as np

B, S, D, H = 4, 2048, 1024, 64
P = 128
CH = 512
KSUB = D // P
NCH = S // CH
NBLK = CH // P
WS = 32.0
SC = 1.0 / (WS * WS * np.sqrt(H))
XB_COLS = 256

_NC = None
TRACE = False
LAST = {}


def build_bass():
    import concourse.bass as bass  # noqa: F401
    import concourse.mybir as mybir
    import concourse.tile as tile
    from concourse import bacc

    f32 = mybir.dt.float32
    u8 = mybir.dt.uint8
    bf16 = mybir.dt.bfloat16
    fp8 = mybir.dt.float8e4
    AF = mybir.ActivationFunctionType
    DR = mybir.MatmulPerfMode.DoubleRow
    ALU = mybir.AluOpType

    nc = bacc.Bacc()
    xa_d = nc.dram_tensor("xa", [D, S], fp8, kind="ExternalInput")
    xb_d = nc.dram_tensor("xb", [P, KSUB * XB_COLS], fp8, kind="ExternalInput")
    wall_d = nc.dram_tensor("wall", [P, 2 * KSUB * P], fp8, kind="ExternalInput")
    bt_d = nc.dram_tensor("bt", [P, 8], f32, kind="ExternalInput")
    out_d = nc.dram_tensor("out", [P, 2 * CH], f32, kind="ExternalOutput")
    den_d = nc.dram_tensor("den", [64, CH], f32, kind="ExternalOutput")

    xa_r = xa_d.rearrange("(o p) s -> p o s", p=P)
    wk_r = wall_d[:, 0:KSUB * H].rearrange("p (o m) -> p o m", m=H)
    wq_r = wall_d[:, KSUB * H:KSUB * P].rearrange("p (o m) -> p o m", m=H)
    wvm_r = wall_d[:, KSUB * P:].rearrange("p (o m) -> p o m", m=P)

    with tile.TileContext(nc) as tc:
        with (
            tc.tile_pool(name="consts", bufs=1) as consts,
            tc.tile_pool(name="data", bufs=1) as data,
            tc.tile_pool(name="ptp", bufs=6) as ptpool,
            tc.tile_pool(name="ptbp", bufs=2) as ptbpool,
            tc.tile_pool(name="vbfp", bufs=3) as vbfpool,
            tc.tile_pool(name="scp", bufs=2, space="PSUM") as scpool,
            tc.tile_pool(name="pjp", bufs=1, space="PSUM") as pjpool,
            tc.tile_pool(name="pop", bufs=1, space="PSUM") as popp,
            tc.tile_pool(name="pdp", bufs=1, space="PSUM") as pdp,
        ):
            wk = consts.tile([P, KSUB, H], fp8)
            wq = consts.tile([P, KSUB, H], fp8)
            wvm = consts.tile([P, KSUB, P], fp8)
            bt = consts.tile([P, 8], f32)
            # ones2[s]: DR denom stationary; slot s's ones in cols 32s:32s+32
            ones2 = [consts.tile([P, 2, 64], fp8, tag=f"ones{s}", name=f"ones{s}")
                     for s in range(2)]
            onesbA = consts.tile([P, 64], bf16)
            zs = consts.tile([P, CH], bf16)
            prime = consts.tile([P, 1], f32)

            xa = [data.tile([P, KSUB, CH], fp8, tag=f"xa{c}", name=f"xa{c}")
                  for c in range(NCH)]
            xb0 = data.tile([P, KSUB, XB_COLS], fp8, tag="xb0", name="xb0")
            kt = [data.tile([64, CH], bf16, tag=f"kt{c}", name=f"kt{c}")
                  for c in range(NCH)]
            qd = [data.tile([64, CH], bf16, tag=f"qd{s}", name=f"qd{s}")
                  for s in range(2)]
            vext = [data.tile([P, NBLK, P], fp8, tag=f"vx{c}", name=f"vx{c}")
                    for c in range(NCH)]
            outsb = data.tile([P, 2, CH], f32, tag="outsb", name="outsb")
            outd = data.tile([64, CH], f32, tag="outd", name="outd")

            # --- DMAs
            nc.sync.dma_start(xa[0][:], xa_r[:, :, 0:CH])
            nc.sync.dma_start(wk[:], wk_r[:])
            nc.sync.dma_start(wq[:], wq_r[:])
            nc.sync.dma_start(bt[:], bt_d[:])
            nc.sync.dma_start(xa[1][:], xa_r[:, :, CH:2 * CH])
            nc.sync.dma_start(wvm[:], wvm_r[:])
            nc.sync.dma_start(xb0[:], xb_d.rearrange("p (o m) -> p o m", m=XB_COLS))
            nc.sync.dma_start(xa[2][:], xa_r[:, :, 2 * CH:3 * CH])
            nc.sync.dma_start(xa[3][:], xa_r[:, :, 3 * CH:4 * CH])

            # --- consts + primers
            for s in range(2):
                nc.gpsimd.memset(ones2[s][:], 0.0)
                nc.gpsimd.memset(ones2[s][:, :, 32 * s:32 * s + 32], 1.0)
            nc.gpsimd.memset(onesbA[:], 0.0)
            nc.gpsimd.memset(onesbA[:, 0:32], 1.0)
            nc.gpsimd.memset(zs[:], 0.0)
            nc.gpsimd.memset(prime[:], 0.25)
            nc.scalar.activation(prime[:], prime[:], AF.Exp)
            nc.vector.tensor_copy(out=prime[:], in_=bt[:, 0:1])
            nc.scalar.activation(prime[:], bt[:, 1:2], AF.Exp)

            po = popp.tile([P, 2, CH], f32, tag="po", name="po")
            pd = pdp.tile([64, CH], f32, tag="pd", name="pd")

            # --- PE warm-up (po/pd zero-init happens after startup projs)
            warm = pjpool.tile([P, CH], f32, tag="pj", name="warm")
            for i in range(5):
                nc.tensor.matmul(warm[0:64, :], zs[:, 0:64], zs[:],
                                 start=True, stop=True)

            def zero_accs():
                # a start=True matmul marks the full 2KB bank region
                # pending-zero on the partitions it covers, so 64-col
                # dummies (27ns) zero the whole accumulators.
                for s in range(2):
                    nc.tensor.matmul(po[:, s, 0:64], zs[:, 0:128], zs[:, 0:64],
                                     start=True, stop=False,
                                     skip_group_check=True)
                nc.tensor.matmul(pd[:, 0:64], zs[:, 0:64], zs[:, 0:64],
                                 start=True, stop=False, skip_group_check=True)

            def fin(last=False):
                return dict(start=False, stop=last, skip_group_check=True)

            vbf = {}

            # --- building blocks --------------------------------------
            def proj_k(c, ps=None, bias=True):
                t = ps if ps is not None else pjpool.tile(
                    [P, CH], f32, tag="pj", name=f"psK{c}")
                for o in range(4):
                    nc.tensor.matmul(
                        t[0:64, :], wk[:, 2 * o:2 * o + 2, :],
                        xa[c][:, 2 * o:2 * o + 2, :],
                        start=(o == 0), stop=(o == 3), perf_mode=DR)
                if bias:
                    nc.vector.tensor_scalar_add(
                        kt[c][:], t[0:64, :], bt[0:64, 1:2])
                return t

            def bias_k(c, t):
                nc.vector.tensor_scalar_add(kt[c][:], t[0:64, :], bt[0:64, 1:2])

            def proj_q(c, s, ps=None, bias=True):
                t = ps if ps is not None else pjpool.tile(
                    [P, CH], f32, tag="pj", name=f"psQ{c}")
                for o in range(4):
                    nc.tensor.matmul(
                        t[0:64, :], wq[:, 2 * o:2 * o + 2, :],
                        xa[c][:, 2 * o:2 * o + 2, :],
                        start=(o == 0), stop=(o == 3), perf_mode=DR)
                if bias:
                    nc.vector.tensor_scalar_add(
                        qd[s][:], t[0:64, :], bt[0:64, 2:3])
                return t

            def bias_q(s, t):
                nc.vector.tensor_scalar_add(qd[s][:], t[0:64, :], bt[0:64, 2:3])

            def proj_v(c):
                t = pjpool.tile([P, CH], f32, tag="pj", name=f"psV{c}")
                for b in range(NBLK):
                    gi = 0
                    for half in range(2):
                        for o in range(4):
                            nc.tensor.matmul(
                                t[:, b * H:(b + 1) * H],
                                xa[c][:, 2 * o:2 * o + 2, b * P:(b + 1) * P],
                                wvm[:, 2 * o:2 * o + 2, half * H:(half + 1) * H],
                                start=(gi == 0), stop=(gi == 7), perf_mode=DR)
                            gi += 1
                tb = vbfpool.tile([P, NBLK, H], bf16, tag="vbf", name=f"vbf{c}")
                vbf[c] = tb
                nc.vector.tensor_copy(
                    out=tb[:], in_=t[:, 0:NBLK * H].rearrange(
                        "p (b m) -> p b m", m=H))

            def corr_v():
                t = pjpool.tile([P, CH], f32, tag="pj", name="psC")
                for b in range(2):
                    gi = 0
                    for half in range(2):
                        for o in range(4):
                            nc.tensor.matmul(
                                t[:, b * H:(b + 1) * H],
                                xb0[:, 2 * o:2 * o + 2, b * P:(b + 1) * P],
                                wvm[:, 2 * o:2 * o + 2, half * H:(half + 1) * H],
                                start=(gi == 0), stop=(gi == 7), perf_mode=DR)
                            gi += 1
                cs = t[:, 0:2 * H].rearrange("p (b m) -> p b m", m=H)
                nc.vector.tensor_tensor(
                    out=vbf[0][:, 0:2, :], in0=cs[:],
                    in1=vbf[0][:, 0:2, :], op=ALU.add)

            def v_quant(c):
                nc.vector.tensor_copy(out=vext[c][:, :, 0:64], in_=vbf[c][:])
                nc.vector.tensor_tensor(
                    out=vext[c][:, :, 64:128], in0=vbf[c][:],
                    in1=vext[c][:, :, 0:64], op=ALU.subtract)

            def pair_scores(s, c, t, kind, sc):
                scf = sc.rearrange("p f c -> p (f c)")
                if kind == "d1":
                    nc.tensor.matmul(scf[:, 0:256],
                                     kt[c][:, 2 * t * P:(2 * t + 1) * P],
                                     qd[s][:, 256:512], start=True, stop=True)
                    nc.tensor.matmul(scf[:, 256:384],
                                     kt[c][:, (2 * t + 1) * P:(2 * t + 2) * P],
                                     qd[s][:, 384:512], start=False, stop=True,
                                     skip_group_check=True)
                elif kind == "d0":
                    nc.tensor.matmul(sc[:, 0, :], kt[c][:, 0:P], qd[s][:],
                                     start=True, stop=True)
                    nc.tensor.matmul(scf[:, 512:896],
                                     kt[c][:, P:2 * P], qd[s][:, 128:512],
                                     start=True, stop=True)
                else:
                    for j in range(2):
                        nc.tensor.matmul(
                            sc[:, j, :],
                            kt[c][:, (2 * t + j) * P:(2 * t + j + 1) * P],
                            qd[s][:], start=True, stop=True)

            def pair_exp(pt, sc, kind, bcol):
                ptf = pt.rearrange("p f c -> p (f c)")
                scf = sc.rearrange("p f c -> p (f c)")
                bias = bt[:, bcol:bcol + 1]
                if kind == "d1":
                    nc.scalar.activation(ptf[:, 0:384], scf[:, 0:384],
                                         AF.Exp, bias=bias, scale=SC)
                elif kind == "d0":
                    nc.scalar.activation(ptf[:, 0:896], scf[:, 0:896],
                                         AF.Exp, bias=bias, scale=SC)
                else:
                    nc.scalar.activation(pt[:], sc[:],
                                         AF.Exp, bias=bias, scale=SC)

            def pair_mask(pt, kind):
                ptf = pt.rearrange("p f c -> p (f c)")
                wins = [(0, 512), (512, 384)] if kind == "d0" else [(0, 256), (256, 128)]
                for off, w in wins:
                    nc.gpsimd.affine_select(
                        out=ptf[:, off:off + w], in_=ptf[:, off:off + w],
                        compare_op=ALU.is_ge, fill=0.0,
                        base=0, pattern=[[1, w]], channel_multiplier=-1)

            def pair_av(s, c, t, kind, use_bf, pt, last=False, part=None):
                ptf = pt.rearrange("p f c -> p (f c)")
                if kind == "full":
                    if last:
                        nc.tensor.matmul(pd[:], ones2[s][:], pt[:],
                                         perf_mode=DR, **fin(last))
                        nc.tensor.matmul(po[:, s, :],
                                         vext[c][:, 2 * t:2 * t + 2, :],
                                         pt[:], perf_mode=DR, **fin(last))
                    else:
                        nc.tensor.matmul(po[:, s, :],
                                         vext[c][:, 2 * t:2 * t + 2, :],
                                         pt[:], perf_mode=DR, **fin(last))
                        nc.tensor.matmul(pd[:], ones2[s][:], pt[:],
                                         perf_mode=DR, **fin(last))
                    return
                if use_bf:
                    # plain bf16 path (slot0 diag): stationary vbf0 / onesbA
                    if kind == "d0":
                        mms = [(0, (0, 512), (0, 512)), (1, (512, 896), (128, 512))]
                    else:
                        mms = [(2, (0, 256), (256, 512)), (3, (256, 384), (384, 512))]
                    if part is not None:
                        mms = [mms[part]]
                    for blk, (plo, phi), (qlo, qhi) in mms:
                        nc.tensor.matmul(po[0:64, s, qlo:qhi], vbf[0][:, blk, :],
                                         ptf[:, plo:phi], **fin(False))
                        nc.tensor.matmul(pd[0:64, qlo:qhi], onesbA[:],
                                         ptf[:, plo:phi], **fin(False))
                    return
                # fp8 trimmed path (slot1 diag): DR common window + plain solo
                if kind == "d0":
                    com = ptf[:, 128:896].rearrange("p (f c) -> p f c", f=2)
                    qlo, qhi, slo, shi = 128, 512, 0, 128
                else:
                    com = ptf[:, 128:384].rearrange("p (f c) -> p f c", f=2)
                    qlo, qhi, slo, shi = 384, 512, 256, 384
                nc.tensor.matmul(po[:, s, qlo:qhi], vext[c][:, 2 * t:2 * t + 2, :],
                                 com, perf_mode=DR, **fin(False))
                nc.tensor.matmul(pd[:, qlo:qhi], ones2[s][:], com,
                                 perf_mode=DR, **fin(False))
                nc.tensor.matmul(po[:, s, slo:shi], vext[c][:, 2 * t, :],
                                 ptf[:, 0:128], **fin(False))
                nc.tensor.matmul(pd[0:64, slo:shi], ones2[s][:, 0, :],
                                 ptf[:, 0:128], **fin(False))

            pts = {}

            FE = 11.5416  # 8/ln2: fp8-e4m3 bit-trick exp slope

            def emit_pair(name, s, c, t, kind, use_bf, bcol, do_mask,
                          dve_exp=False):
                sc = scpool.tile([P, 2, CH], f32, tag="sc", name=f"sc_{name}")
                pair_scores(s, c, t, kind, sc)
                if use_bf:
                    pt = ptbpool.tile([P, 2, CH], bf16, tag="ptb", name=f"pt_{name}")
                else:
                    pt = ptpool.tile([P, 2, CH], fp8, tag="pt", name=f"pt_{name}")
                pts[name] = (pt, s, c, t, kind, use_bf)
                if dve_exp:
                    # piecewise-linear exp via float->uint8 convert: bits =
                    # round(s*SC*FE + 56 + theta*FE); negatives saturate to
                    # fp8 +0 (exact gating for masked pairs).
                    nc.vector.tensor_scalar(
                        out=pt.bitcast(u8)[:], in0=sc[:], scalar1=SC * FE,
                        scalar2=bt[:, bcol:bcol + 1],
                        op0=ALU.mult, op1=ALU.add)
                else:
                    pair_exp(pt, sc, kind, bcol)
                if do_mask:
                    pair_mask(pt, kind)

            def emit_av(name, last=False, part=None):
                pt, s, c, t, kind, use_bf = pts[name]
                pair_av(s, c, t, kind, use_bf, pt, last=last, part=part)

            # --- program ------------------------------------------------
            # Startup: K0/Q0 with hi-half biases (ACT + DVE in parallel) so
            # p2 (d1, hi-half only) fires early; K1/Q1 matmuls staged before
            # p1 with their biases after the low-half biases.
            psQ0 = scpool.tile([P, 2, CH], f32, tag="sc", name="psQ0")
            psK0 = pjpool.tile([P, CH], f32, tag="pj", name="psK0")
            for o in range(4):
                nc.tensor.matmul(psK0[0:64, :], wk[:, 2 * o:2 * o + 2, :],
                                 xa[0][:, 2 * o:2 * o + 2, :],
                                 start=(o == 0), stop=(o == 3), perf_mode=DR)
            for o in range(4):
                nc.tensor.matmul(psQ0[0:64, 0, :], wq[:, 2 * o:2 * o + 2, :],
                                 xa[0][:, 2 * o:2 * o + 2, :],
                                 start=(o == 0), stop=(o == 3), perf_mode=DR)
            nc.scalar.activation(kt[0][:, 256:512], psK0[0:64, 256:512],
                                 AF.Identity, bias=bt[0:64, 1:2])
            nc.vector.tensor_scalar_add(
                qd[0][:, 256:512], psQ0[0:64, 0, 256:512], bt[0:64, 2:3])
            zero_accs()
            emit_pair("p2", 0, 0, 1, "d1", True, 0, True)
            psK1 = proj_k(1, bias=False)
            psQ1 = scpool.tile([P, 2, CH], f32, tag="sc", name="psQ1")
            proj_q(1, 1, ps=psQ1[:, 0, :], bias=False)
            nc.scalar.activation(kt[0][:, 0:256], psK0[0:64, 0:256],
                                 AF.Identity, bias=bt[0:64, 1:2])
            nc.vector.tensor_scalar_add(
                qd[0][:, 0:256], psQ0[0:64, 0, 0:256], bt[0:64, 2:3])
            emit_pair("p1", 0, 0, 0, "d0", True, 0, True)
            nc.vector.tensor_scalar_add(
                qd[1][:], psQ1[0:64, 0, :], bt[0:64, 2:3])
            nc.vector.tensor_scalar_add(
                kt[1][:, 0:256], psK1[0:64, 0:256], bt[0:64, 1:2])
            emit_pair("p3", 1, 0, 0, "full", False, 0, False)
            nc.vector.tensor_scalar_add(
                kt[1][:, 256:512], psK1[0:64, 256:512], bt[0:64, 1:2])
            proj_k(2)
            proj_v(0)
            corr_v()
            v_quant(0)
            emit_pair("p4", 1, 0, 1, "full", False, 0, False)
            emit_pair("p5", 1, 1, 0, "d0", False, 0, True)
            emit_av("p3")
            proj_v(1)
            v_quant(1)
            emit_pair("p6", 1, 1, 1, "d1", False, 0, True)
            emit_av("p4")
            proj_k(3)
            emit_av("p2", part=0)
            emit_pair("p7", 0, 2, 0, "full", False, 3, False)
            emit_av("p2", part=1)
            emit_av("p1", part=0)
            emit_pair("p8", 0, 2, 1, "full", False, 3, False)
            emit_av("p1", part=1)
            emit_pair("p9", 1, 2, 0, "full", False, 0, False)
            emit_av("p5")
            proj_v(2)
            v_quant(2)
            emit_pair("p10", 1, 2, 1, "full", False, 0, False)
            emit_av("p6")
            emit_av("p7")
            emit_av("p8", last=True)
            # slot0 accumulators complete: flush
            nc.vector.tensor_copy(out=outsb[:, 0, :], in_=po[:, 0, :])
            nc.vector.tensor_copy(out=outd[0:32, :], in_=pd[0:32, :])
            nc.sync.dma_start(out_d[:, 0:CH], outsb[:, 0, :])
            proj_v(3)
            v_quant(3)
            emit_pair("p11", 1, 3, 0, "full", False, 4, False)
            emit_pair("p12", 1, 3, 1, "full", False, 4, False)
            emit_av("p9")
            emit_av("p10")
            emit_av("p11")
            emit_av("p12", last=True)
            nc.scalar.copy(outd[32:64, :], pd[32:64, :])
            nc.vector.tensor_copy(out=outsb[:, 1, :], in_=po[:, 1, :])
            nc.sync.dma_start(out_d[:, CH:2 * CH], outsb[:, 1, :])
            nc.sync.dma_start(den_d[:], outd[:])

    nc.compile()
    return nc


def make_in_maps(x, Wq, bq, Wk, bk, Wv, bv):
    import ml_dtypes
    e4 = ml_dtypes.float8_e4m3
    x = np.asarray(x, dtype=np.float32)
    wk8 = (np.asarray(Wk, np.float32) * WS).astype(e4)
    wq8 = (np.asarray(Wq, np.float32) * WS).astype(e4)
    wva = (np.asarray(Wv, np.float32) * WS).astype(e4)
    wvb = (np.asarray(Wv, np.float32) * WS - wva.astype(np.float32)).astype(e4)
    wkh = wk8.reshape(KSUB, P, H).transpose(1, 0, 2).reshape(P, KSUB * H)
    wqh = wq8.reshape(KSUB, P, H).transpose(1, 0, 2).reshape(P, KSUB * H)
    wvm = np.concatenate([wva.reshape(KSUB, P, H), wvb.reshape(KSUB, P, H)],
                         axis=2).transpose(1, 0, 2).reshape(P, KSUB * P)
    wall = np.ascontiguousarray(np.concatenate([wkh, wqh, wvm], axis=1))
    in_maps = []
    for c in range(8):
        b, g = c // 2, c % 2
        A, Bc, r0, r1 = ((0, 3, 1, 2) if g == 0 else (1, 2, 0, 3))
        perm = np.concatenate([np.arange(cc * CH, (cc + 1) * CH)
                               for cc in (A, Bc, r0, r1)])
        xT = np.ascontiguousarray(x[b][perm].T)
        xa = xT.astype(e4)
        xbr = (xT[:, :XB_COLS] - xa[:, :XB_COLS].astype(np.float32)).astype(e4)
        # pack xb partition-major contiguous: [P, KSUB*XB_COLS]
        xb = np.ascontiguousarray(
            xbr.reshape(KSUB, P, XB_COLS).transpose(1, 0, 2).reshape(
                P, KSUB * XB_COLS))
        bt = np.zeros((P, 8), np.float32)
        bt[0:64, 1] = np.asarray(bk, np.float32) * WS
        bt[0:64, 2] = np.asarray(bq, np.float32) * WS
        bt[:, 3] = 0.0 if r0 < A else -40.0
        bt[:, 4] = 0.0 if r1 < Bc else -40.0
        bt[:, 5] = 56.0 + 11.5416 * bt[0, 3]
        bt[:, 6] = 56.0 + 11.5416 * bt[0, 4]
        bt[:, 7] = 56.0
        in_maps.append({"xa": xa, "xb": np.ascontiguousarray(xb),
                        "wall": wall, "bt": bt})
    return in_maps


def gather(results, bv):
    bv = np.asarray(bv, np.float32)
    out = np.zeros((B, S, H), np.float32)
    for c in range(8):
        b, g = c // 2, c % 2
        A, Bc = (0, 3) if g == 0 else (1, 2)
        r = results[c]["out"]
        d = results[c]["den"]
        for s, cc in ((0, A), (1, Bc)):
            num = r[0:H, s * CH:(s + 1) * CH] + r[H:2 * H, s * CH:(s + 1) * CH]
            o = (num / d[32 * s]) / WS
            out[b, cc * CH:(cc + 1) * CH] = o.T + bv
    return out


def kernel(x, Wq, bq, Wk, bk, Wv, bv):
    global _NC
    from concourse.bass_utils import run_bass_kernel_spmd

    if _NC is None:
        _NC = build_bass()
    in_maps = make_in_maps(x, Wq, bq, Wk, bk, Wv, bv)
    res = run_bass_kernel_spmd(_NC, in_maps, core_ids=list(range(8)), trace=TRACE)
    LAST["res"] = res
    return gather(res.results, bv)


# revision 55
# speedup vs baseline: 1.0135x; 1.0135x over previous
"""Causal single-head attention (b=4, s=2048, d=1024, h=64) on 8 TRN2 cores.

Sharding: core c -> (batch b = c//2, g = c%2); the core owns the balanced
q-chunk pair A,B = (0,3) if g==0 else (1,2) (512 queries each).  Permuted
row order [A; B; rest0; rest1] makes one uniform SPMD program valid for
all 8 cores (cross-chunk gating via exp biases in {0, -40}).

fp8 DoubleRow pipeline:
  - x is shipped as fp8 xa (+ an fp8 residual xb0 for the first 256 keys
    of chunk A, protecting the low-N_eff early rows' v precision).
  - K/Q projected with DR matmuls to PSUM partitions 0:64 (kt/qd bf16,
    weights host-scaled x32; the combined scale lands in the exp scale;
    bv is folded in on the host after the division).
  - V projected directly in transposed [keys, h] layout (x stationary,
    Wva+Wvb double-quantized fp8 weights accumulated into one psum), then
    evacuated to bf16 vbf and double-quantized to fp8 a+b in SBUF ->
    vext [128k, blk, a|b].
  - scores bf16; one exp per 2-k-block pair (f32 psum -> fp8 pt; bf16 pt
    for the precision-critical slot0-diag pairs p1/p2, which also use
    bf16 plain AV matmuls against vbf0).  Diag pairs trim the causal
    triangle: pair t=0 widths (512,384), t=1 (256,128), packed flat.
  - diag masks via gpsimd affine_select on pt (keep p<=f).
  - AV: fp8 DR matmul per pair -> po[s]=[A;B] partials; denominators via
    DR ones-matmuls into ONE pd bank (slot0's ones live in stationary
    cols 0:32, slot1's in 32:64, each zero-padded so both slots share
    the [64, 512] accumulator).  Host: out = (A+B)/den/32 + bv.
  - PSUM: sc pool (2 x 2-bank score-pair tiles, double-buffered) + pj
    pool (1 bank serializing K/V/corr projections; Q borrows sc bufs) +
    po (2 banks) + pd (1) = 8 banks.
  - PE warm-up matmuls hold the p-state ramp until the first x DMA lands.
  - Software-pipelined emission: scores run 2 pairs ahead of AVs so the
    in-order PE queue never blocks on exp latency.
"""

import numpy as np

B, S, D, H = 4, 2048, 1024, 64
P = 128
CH = 512
KSUB = D // P
NCH = S // CH
NBLK = CH // P
WS = 32.0
SC = 1.0 / (WS * WS * np.sqrt(H))
XB_COLS = 256

_NC = None
TRACE = False
LAST = {}


def build_bass():
    import concourse.bass as bass  # noqa: F401
    import concourse.mybir as mybir
    import concourse.tile as tile
    from concourse import bacc

    f32 = mybir.dt.float32
    u8 = mybir.dt.uint8
    bf16 = mybir.dt.bfloat16
    fp8 = mybir.dt.float8e4
    AF = mybir.ActivationFunctionType
    DR = mybir.MatmulPerfMode.DoubleRow
    ALU = mybir.AluOpType

    nc = bacc.Bacc()
    xa_d = nc.dram_tensor("xa", [D, S], fp8, kind="ExternalInput")
    xb_d = nc.dram_tensor("xb", [P, KSUB * XB_COLS], fp8, kind="ExternalInput")
    wall_d = nc.dram_tensor("wall", [P, 2 * KSUB * P], fp8, kind="ExternalInput")
    bt_d = nc.dram_tensor("bt", [P, 8], f32, kind="ExternalInput")
    out_d = nc.dram_tensor("out", [P, 2 * CH], f32, kind="ExternalOutput")
    den_d = nc.dram_tensor("den", [64, CH], f32, kind="ExternalOutput")

    xa_r = xa_d.rearrange("(o p) s -> p o s", p=P)
    wk_r = wall_d[:, 0:KSUB * H].rearrange("p (o m) -> p o m", m=H)
    wq_r = wall_d[:, KSUB * H:KSUB * P].rearrange("p (o m) -> p o m", m=H)
    wvm_r = wall_d[:, KSUB * P:].rearrange("p (o m) -> p o m", m=P)

    with tile.TileContext(nc) as tc:
        with (
            tc.tile_pool(name="consts", bufs=1) as consts,
            tc.tile_pool(name="data", bufs=1) as data,
            tc.tile_pool(name="ptp", bufs=6) as ptpool,
            tc.tile_pool(name="ptbp", bufs=2) as ptbpool,
            tc.tile_pool(name="vbfp", bufs=3) as vbfpool,
            tc.tile_pool(name="scp", bufs=2, space="PSUM") as scpool,
            tc.tile_pool(name="pjp", bufs=1, space="PSUM") as pjpool,
            tc.tile_pool(name="pop", bufs=1, space="PSUM") as popp,
            tc.tile_pool(name="pdp", bufs=1, space="PSUM") as pdp,
        ):
            wk = consts.tile([P, KSUB, H], fp8)
            wq = consts.tile([P, KSUB, H], fp8)
            wvm = consts.tile([P, KSUB, P], fp8)
            bt = consts.tile([P, 8], f32)
            # ones2[s]: DR denom stationary; slot s's ones in cols 32s:32s+32
            ones2 = [consts.tile([P, 2, 64], fp8, tag=f"ones{s}", name=f"ones{s}")
                     for s in range(2)]
            onesbA = consts.tile([P, 64], bf16)
            zs = consts.tile([P, CH], bf16)
            prime = consts.tile([P, 1], f32)

            xa = [data.tile([P, KSUB, CH], fp8, tag=f"xa{c}", name=f"xa{c}")
                  for c in range(NCH)]
            xb0 = data.tile([P, KSUB, XB_COLS], fp8, tag="xb0", name="xb0")
            kt = [data.tile([64, CH], bf16, tag=f"kt{c}", name=f"kt{c}")
                  for c in range(NCH)]
            qd = [data.tile([64, CH], bf16, tag=f"qd{s}", name=f"qd{s}")
                  for s in range(2)]
            vext = [data.tile([P, NBLK, P], fp8, tag=f"vx{c}", name=f"vx{c}")
                    for c in range(NCH)]
            outsb = data.tile([P, 2, CH], f32, tag="outsb", name="outsb")
            outd = data.tile([64, CH], f32, tag="outd", name="outd")

            # --- DMAs
            nc.sync.dma_start(xa[0][:], xa_r[:, :, 0:CH])
            nc.sync.dma_start(wk[:], wk_r[:])
            nc.sync.dma_start(wq[:], wq_r[:])
            nc.sync.dma_start(bt[:], bt_d[:])
            nc.sync.dma_start(xa[1][:], xa_r[:, :, CH:2 * CH])
            nc.sync.dma_start(wvm[:], wvm_r[:])
            nc.sync.dma_start(xb0[:], xb_d.rearrange("p (o m) -> p o m", m=XB_COLS))
            nc.sync.dma_start(xa[2][:], xa_r[:, :, 2 * CH:3 * CH])
            nc.sync.dma_start(xa[3][:], xa_r[:, :, 3 * CH:4 * CH])

            # --- consts + primers
            for s in range(2):
                nc.gpsimd.memset(ones2[s][:], 0.0)
                nc.gpsimd.memset(ones2[s][:, :, 32 * s:32 * s + 32], 1.0)
            nc.gpsimd.memset(onesbA[:], 0.0)
            nc.gpsimd.memset(onesbA[:, 0:32], 1.0)
            nc.gpsimd.memset(zs[:], 0.0)
            nc.gpsimd.memset(prime[:], 0.25)
            nc.scalar.activation(prime[:], prime[:], AF.Exp)
            nc.vector.tensor_copy(out=prime[:], in_=bt[:, 0:1])
            nc.scalar.activation(prime[:], bt[:, 1:2], AF.Exp)

            po = popp.tile([P, 2, CH], f32, tag="po", name="po")
            pd = pdp.tile([64, CH], f32, tag="pd", name="pd")

            # --- PE warm-up (po/pd zero-init happens after startup projs)
            warm = pjpool.tile([P, CH], f32, tag="pj", name="warm")
            for i in range(5):
                nc.tensor.matmul(warm[0:64, :], zs[:, 0:64], zs[:],
                                 start=True, stop=True)

            def zero_accs():
                # a start=True matmul marks the full 2KB bank region
                # pending-zero on the partitions it covers, so 64-col
                # dummies (27ns) zero the whole accumulators.
                for s in range(2):
                    nc.tensor.matmul(po[:, s, 0:64], zs[:, 0:128], zs[:, 0:64],
                                     start=True, stop=False,
                                     skip_group_check=True)
                nc.tensor.matmul(pd[:, 0:64], zs[:, 0:64], zs[:, 0:64],
                                 start=True, stop=False, skip_group_check=True)

            def fin(last=False):
                return dict(start=False, stop=last, skip_group_check=True)

            vbf = {}

            # --- building blocks --------------------------------------
            def proj_k(c, ps=None, bias=True):
                t = ps if ps is not None else pjpool.tile(
                    [P, CH], f32, tag="pj", name=f"psK{c}")
                for o in range(4):
                    nc.tensor.matmul(
                        t[0:64, :], wk[:, 2 * o:2 * o + 2, :],
                        xa[c][:, 2 * o:2 * o + 2, :],
                        start=(o == 0), stop=(o == 3), perf_mode=DR)
                if bias:
                    nc.vector.tensor_scalar_add(
                        kt[c][:], t[0:64, :], bt[0:64, 1:2])
                return t

            def bias_k(c, t):
                nc.vector.tensor_scalar_add(kt[c][:], t[0:64, :], bt[0:64, 1:2])

            def proj_q(c, s, ps=None, bias=True):
                t = ps if ps is not None else pjpool.tile(
                    [P, CH], f32, tag="pj", name=f"psQ{c}")
                for o in range(4):
                    nc.tensor.matmul(
                        t[0:64, :], wq[:, 2 * o:2 * o + 2, :],
                        xa[c][:, 2 * o:2 * o + 2, :],
                        start=(o == 0), stop=(o == 3), perf_mode=DR)
                if bias:
                    nc.vector.tensor_scalar_add(
                        qd[s][:], t[0:64, :], bt[0:64, 2:3])
                return t

            def bias_q(s, t):
                nc.vector.tensor_scalar_add(qd[s][:], t[0:64, :], bt[0:64, 2:3])

            def proj_v(c):
                t = pjpool.tile([P, CH], f32, tag="pj", name=f"psV{c}")
                for b in range(NBLK):
                    gi = 0
                    for half in range(2):
                        for o in range(4):
                            nc.tensor.matmul(
                                t[:, b * H:(b + 1) * H],
                                xa[c][:, 2 * o:2 * o + 2, b * P:(b + 1) * P],
                                wvm[:, 2 * o:2 * o + 2, half * H:(half + 1) * H],
                                start=(gi == 0), stop=(gi == 7), perf_mode=DR)
                            gi += 1
                tb = vbfpool.tile([P, NBLK, H], bf16, tag="vbf", name=f"vbf{c}")
                vbf[c] = tb
                nc.vector.tensor_copy(
                    out=tb[:], in_=t[:, 0:NBLK * H].rearrange(
                        "p (b m) -> p b m", m=H))

            def corr_v():
                t = pjpool.tile([P, CH], f32, tag="pj", name="psC")
                for b in range(2):
                    gi = 0
                    for half in range(2):
                        for o in range(4):
                            nc.tensor.matmul(
                                t[:, b * H:(b + 1) * H],
                                xb0[:, 2 * o:2 * o + 2, b * P:(b + 1) * P],
                                wvm[:, 2 * o:2 * o + 2, half * H:(half + 1) * H],
                                start=(gi == 0), stop=(gi == 7), perf_mode=DR)
                            gi += 1
                cs = t[:, 0:2 * H].rearrange("p (b m) -> p b m", m=H)
                nc.vector.tensor_tensor(
                    out=vbf[0][:, 0:2, :], in0=cs[:],
                    in1=vbf[0][:, 0:2, :], op=ALU.add)

            def v_quant(c):
                nc.vector.tensor_copy(out=vext[c][:, :, 0:64], in_=vbf[c][:])
                nc.vector.tensor_tensor(
                    out=vext[c][:, :, 64:128], in0=vbf[c][:],
                    in1=vext[c][:, :, 0:64], op=ALU.subtract)

            def pair_scores(s, c, t, kind, sc):
                scf = sc.rearrange("p f c -> p (f c)")
                if kind == "d1":
                    nc.tensor.matmul(scf[:, 0:256],
                                     kt[c][:, 2 * t * P:(2 * t + 1) * P],
                                     qd[s][:, 256:512], start=True, stop=True)
                    nc.tensor.matmul(scf[:, 256:384],
                                     kt[c][:, (2 * t + 1) * P:(2 * t + 2) * P],
                                     qd[s][:, 384:512], start=False, stop=True,
                                     skip_group_check=True)
                elif kind == "d0":
                    nc.tensor.matmul(sc[:, 0, :], kt[c][:, 0:P], qd[s][:],
                                     start=True, stop=True)
                    nc.tensor.matmul(scf[:, 512:896],
                                     kt[c][:, P:2 * P], qd[s][:, 128:512],
                                     start=True, stop=True)
                else:
                    for j in range(2):
                        nc.tensor.matmul(
                            sc[:, j, :],
                            kt[c][:, (2 * t + j) * P:(2 * t + j + 1) * P],
                            qd[s][:], start=True, stop=True)

            def pair_exp(pt, sc, kind, bcol):
                ptf = pt.rearrange("p f c -> p (f c)")
                scf = sc.rearrange("p f c -> p (f c)")
                bias = bt[:, bcol:bcol + 1]
                if kind == "d1":
                    nc.scalar.activation(ptf[:, 0:384], scf[:, 0:384],
                                         AF.Exp, bias=bias, scale=SC)
                elif kind == "d0":
                    nc.scalar.activation(ptf[:, 0:896], scf[:, 0:896],
                                         AF.Exp, bias=bias, scale=SC)
                else:
                    nc.scalar.activation(pt[:], sc[:],
                                         AF.Exp, bias=bias, scale=SC)

            def pair_mask(pt, kind):
                ptf = pt.rearrange("p f c -> p (f c)")
                wins = [(0, 512), (512, 384)] if kind == "d0" else [(0, 256), (256, 128)]
                for off, w in wins:
                    nc.gpsimd.affine_select(
                        out=ptf[:, off:off + w], in_=ptf[:, off:off + w],
                        compare_op=ALU.is_ge, fill=0.0,
                        base=0, pattern=[[1, w]], channel_multiplier=-1)

            def pair_av(s, c, t, kind, use_bf, pt, last=False, part=None):
                ptf = pt.rearrange("p f c -> p (f c)")
                if kind == "full":
                    nc.tensor.matmul(po[:, s, :], vext[c][:, 2 * t:2 * t + 2, :],
                                     pt[:], perf_mode=DR, **fin(last))
                    nc.tensor.matmul(pd[:], ones2[s][:], pt[:],
                                     perf_mode=DR, **fin(last))
                    return
                if use_bf:
                    # plain bf16 path (slot0 diag): stationary vbf0 / onesbA
                    if kind == "d0":
                        mms = [(0, (0, 512), (0, 512)), (1, (512, 896), (128, 512))]
                    else:
                        mms = [(2, (0, 256), (256, 512)), (3, (256, 384), (384, 512))]
                    if part is not None:
                        mms = [mms[part]]
                    for blk, (plo, phi), (qlo, qhi) in mms:
                        nc.tensor.matmul(po[0:64, s, qlo:qhi], vbf[0][:, blk, :],
                                         ptf[:, plo:phi], **fin(False))
                        nc.tensor.matmul(pd[0:64, qlo:qhi], onesbA[:],
                                         ptf[:, plo:phi], **fin(False))
                    return
                # fp8 trimmed path (slot1 diag): DR common window + plain solo
                if kind == "d0":
                    com = ptf[:, 128:896].rearrange("p (f c) -> p f c", f=2)
                    qlo, qhi, slo, shi = 128, 512, 0, 128
                else:
                    com = ptf[:, 128:384].rearrange("p (f c) -> p f c", f=2)
                    qlo, qhi, slo, shi = 384, 512, 256, 384
                nc.tensor.matmul(po[:, s, qlo:qhi], vext[c][:, 2 * t:2 * t + 2, :],
                                 com, perf_mode=DR, **fin(False))
                nc.tensor.matmul(pd[:, qlo:qhi], ones2[s][:], com,
                                 perf_mode=DR, **fin(False))
                nc.tensor.matmul(po[:, s, slo:shi], vext[c][:, 2 * t, :],
                                 ptf[:, 0:128], **fin(False))
                nc.tensor.matmul(pd[0:64, slo:shi], ones2[s][:, 0, :],
                                 ptf[:, 0:128], **fin(False))

            pts = {}

            FE = 11.5416  # 8/ln2: fp8-e4m3 bit-trick exp slope

            def emit_pair(name, s, c, t, kind, use_bf, bcol, do_mask,
                          dve_exp=False):
                sc = scpool.tile([P, 2, CH], f32, tag="sc", name=f"sc_{name}")
                pair_scores(s, c, t, kind, sc)
                if use_bf:
                    pt = ptbpool.tile([P, 2, CH], bf16, tag="ptb", name=f"pt_{name}")
                else:
                    pt = ptpool.tile([P, 2, CH], fp8, tag="pt", name=f"pt_{name}")
                pts[name] = (pt, s, c, t, kind, use_bf)
                if dve_exp:
                    # piecewise-linear exp via float->uint8 convert: bits =
                    # round(s*SC*FE + 56 + theta*FE); negatives saturate to
                    # fp8 +0 (exact gating for masked pairs).
                    nc.vector.tensor_scalar(
                        out=pt.bitcast(u8)[:], in0=sc[:], scalar1=SC * FE,
                        scalar2=bt[:, bcol:bcol + 1],
                        op0=ALU.mult, op1=ALU.add)
                else:
                    pair_exp(pt, sc, kind, bcol)
                if do_mask:
                    pair_mask(pt, kind)

            def emit_av(name, last=False, part=None):
                pt, s, c, t, kind, use_bf = pts[name]
                pair_av(s, c, t, kind, use_bf, pt, last=last, part=part)

            # --- program ------------------------------------------------
            # Startup: K0/Q0 with hi-half biases (ACT + DVE in parallel) so
            # p2 (d1, hi-half only) fires early; K1/Q1 matmuls staged before
            # p1 with their biases after the low-half biases.
            psQ0 = scpool.tile([P, 2, CH], f32, tag="sc", name="psQ0")
            psK0 = pjpool.tile([P, CH], f32, tag="pj", name="psK0")
            for o in range(4):
                nc.tensor.matmul(psK0[0:64, :], wk[:, 2 * o:2 * o + 2, :],
                                 xa[0][:, 2 * o:2 * o + 2, :],
                                 start=(o == 0), stop=(o == 3), perf_mode=DR)
            for o in range(4):
                nc.tensor.matmul(psQ0[0:64, 0, :], wq[:, 2 * o:2 * o + 2, :],
                                 xa[0][:, 2 * o:2 * o + 2, :],
                                 start=(o == 0), stop=(o == 3), perf_mode=DR)
            nc.scalar.activation(kt[0][:, 256:512], psK0[0:64, 256:512],
                                 AF.Identity, bias=bt[0:64, 1:2])
            nc.vector.tensor_scalar_add(
                qd[0][:, 256:512], psQ0[0:64, 0, 256:512], bt[0:64, 2:3])
            zero_accs()
            emit_pair("p2", 0, 0, 1, "d1", True, 0, True)
            psK1 = proj_k(1, bias=False)
            psQ1 = scpool.tile([P, 2, CH], f32, tag="sc", name="psQ1")
            proj_q(1, 1, ps=psQ1[:, 0, :], bias=False)
            nc.scalar.activation(kt[0][:, 0:256], psK0[0:64, 0:256],
                                 AF.Identity, bias=bt[0:64, 1:2])
            nc.vector.tensor_scalar_add(
                qd[0][:, 0:256], psQ0[0:64, 0, 0:256], bt[0:64, 2:3])
            emit_pair("p1", 0, 0, 0, "d0", True, 0, True)
            nc.vector.tensor_scalar_add(
                qd[1][:], psQ1[0:64, 0, :], bt[0:64, 2:3])
            nc.vector.tensor_scalar_add(
                kt[1][:, 0:256], psK1[0:64, 0:256], bt[0:64, 1:2])
            emit_pair("p3", 1, 0, 0, "full", False, 0, False)
            nc.vector.tensor_scalar_add(
                kt[1][:, 256:512], psK1[0:64, 256:512], bt[0:64, 1:2])
            proj_k(2)
            proj_v(0)
            corr_v()
            v_quant(0)
            emit_pair("p4", 1, 0, 1, "full", False, 0, False)
            emit_pair("p5", 1, 1, 0, "d0", False, 0, True)
            emit_av("p3")
            proj_v(1)
            v_quant(1)
            emit_pair("p6", 1, 1, 1, "d1", False, 0, True)
            emit_av("p4")
            proj_k(3)
            emit_av("p2", part=0)
            emit_pair("p7", 0, 2, 0, "full", False, 3, False)
            emit_av("p2", part=1)
            emit_av("p1", part=0)
            emit_pair("p8", 0, 2, 1, "full", False, 3, False)
            emit_av("p1", part=1)
            emit_pair("p9", 1, 2, 0, "full", False, 0, False)
            emit_av("p5")
            proj_v(2)
            v_quant(2)
            emit_pair("p10", 1, 2, 1, "full", False, 0, False)
            emit_av("p6")
            emit_av("p7")
            emit_av("p8", last=True)
            # slot0 accumulators complete: flush
            nc.vector.tensor_copy(out=outsb[:, 0, :], in_=po[:, 0, :])
            nc.vector.tensor_copy(out=outd[0:32, :], in_=pd[0:32, :])
            nc.sync.dma_start(out_d[:, 0:CH], outsb[:, 0, :])
            proj_v(3)
            v_quant(3)
            emit_pair("p11", 1, 3, 0, "full", False, 4, False)
            emit_pair("p12", 1, 3, 1, "full", False, 4, False)
            emit_av("p9")
            emit_av("p10")
            emit_av("p11")
            emit_av("p12", last=True)
            nc.scalar.copy(outd[32:64, :], pd[32:64, :])
            nc.vector.tensor_copy(out=outsb[:, 1, :], in_=po[:, 1, :])
            nc.sync.dma_start(out_d[:, CH:2 * CH], outsb[:, 1, :])
            nc.sync.dma_start(den_d[:], outd[:])

    nc.compile()
    return nc


def make_in_maps(x, Wq, bq, Wk, bk, Wv, bv):
    import ml_dtypes
    e4 = ml_dtypes.float8_e4m3
    x = np.asarray(x, dtype=np.float32)
    wk8 = (np.asarray(Wk, np.float32) * WS).astype(e4)
    wq8 = (np.asarray(Wq, np.float32) * WS).astype(e4)
    wva = (np.asarray(Wv, np.float32) * WS).astype(e4)
    wvb = (np.asarray(Wv, np.float32) * WS - wva.astype(np.float32)).astype(e4)
    wkh = wk8.reshape(KSUB, P, H).transpose(1, 0, 2).reshape(P, KSUB * H)
    wqh = wq8.reshape(KSUB, P, H).transpose(1, 0, 2).reshape(P, KSUB * H)
    wvm = np.concatenate([wva.reshape(KSUB, P, H), wvb.reshape(KSUB, P, H)],
                         axis=2).transpose(1, 0, 2).reshape(P, KSUB * P)
    wall = np.ascontiguousarray(np.concatenate([wkh, wqh, wvm], axis=1))
    in_maps = []
    for c in range(8):
        b, g = c // 2, c % 2
        A, Bc, r0, r1 = ((0, 3, 1, 2) if g == 0 else (1, 2, 0, 3))
        perm = np.concatenate([np.arange(cc * CH, (cc + 1) * CH)
                               for cc in (A, Bc, r0, r1)])
        xT = np.ascontiguousarray(x[b][perm].T)
        xa = xT.astype(e4)
        xbr = (xT[:, :XB_COLS] - xa[:, :XB_COLS].astype(np.float32)).astype(e4)
        # pack xb partition-major contiguous: [P, KSUB*XB_COLS]
        xb = np.ascontiguousarray(
            xbr.reshape(KSUB, P, XB_COLS).transpose(1, 0, 2).reshape(
                P, KSUB * XB_COLS))
        bt = np.zeros((P, 8), np.float32)
        bt[0:64, 1] = np.asarray(bk, np.float32) * WS
        bt[0:64, 2] = np.asarray(bq, np.float32) * WS
        bt[:, 3] = 0.0 if r0 < A else -40.0
        bt[:, 4] = 0.0 if r1 < Bc else -40.0
        bt[:, 5] = 56.0 + 11.5416 * bt[0, 3]
        bt[:, 6] = 56.0 + 11.5416 * bt[0, 4]
        bt[:, 7] = 56.0
        in_maps.append({"xa": xa, "xb": np.ascontiguousarray(xb),
                        "wall": wall, "bt": bt})
    return in_maps


def gather(results, bv):
    bv = np.asarray(bv, np.float32)
    out = np.zeros((B, S, H), np.float32)
    for c in range(8):
        b, g = c // 2, c % 2
        A, Bc = (0, 3) if g == 0 else (1, 2)
        r = results[c]["out"]
        d = results[c]["den"]
        for s, cc in ((0, A), (1, Bc)):
            num = r[0:H, s * CH:(s + 1) * CH] + r[H:2 * H, s * CH:(s + 1) * CH]
            o = (num / d[32 * s]) / WS
            out[b, cc * CH:(cc + 1) * CH] = o.T + bv
    return out


def kernel(x, Wq, bq, Wk, bk, Wv, bv):
    global _NC
    from concourse.bass_utils import run_bass_kernel_spmd

    if _NC is None:
        _NC = build_bass()
    in_maps = make_in_maps(x, Wq, bq, Wk, bk, Wv, bv)
    res = run_bass_kernel_spmd(_NC, in_maps, core_ids=list(range(8)), trace=TRACE)
    LAST["res"] = res
    return gather(res.results, bv)
